# revision 19
# baseline (speedup 1.0000x reference)
"""Compressed-KV GPT-2 attention block on 8 TRN2 NeuronCores.

Sharding: batch x head-group. Core c: batch b = c//4, heads 4*(c%4)..4*(c%4)+4.
Each core runs the full fused pipeline for its 4 heads in transposed-activation
layout ([dim, seq] on partitions) and emits a partial c_proj output^T; the host
sums the 4 partials per batch and adds b_proj.

Device pipeline per core (all matmuls bf16 -> fp32 PSUM):
  The KV compressor is low-rank and linear, so host folds it:
    k_dec = k @ (wk_c@wk_d)  -> fold W_k into w_attn k-columns (w_k' = w_k W_k)
    v_dec = v @ (wv_c@wv_d)  -> one small on-device matmul with W_v
  qkv^T   = w_qkv^T-chunks @ hidden^T   (m-blocks: q|q, k'|k', v|v head pairs,
            so kdec^T comes straight out of the qkv matmul)
  vdec    = v^T-slices^T @ W_v          (natural [s,d] + ones col for denom)
  S^T     = kdec^T-slices^T @ q^T   -> exp (no-max softmax; causal via mask mul)
  attn^T  = vdec_ones^T @ E (accum over key tiles; row 64 = softmax denom)
  out^T  += w_proj-rows^T @ attn^T  (partial over this core's heads)
"""

import sys

if "/opt/trn_rl_repo" not in sys.path:
    sys.path.insert(0, "/opt/trn_rl_repo")

import numpy as np
import ml_dtypes

BF16 = ml_dtypes.bfloat16

B, S, D = 2, 2048, 1024
H, hd, C = 16, 64, 32
NCORES = 8
HPC = 4            # heads per core
SB = 512           # free-dim block (PSUM bank / max moving cols)
NSB = S // SB      # 4 seq blocks of 512
NKT = S // 128     # 16 key tiles of 128
DC = D // 128      # 8 contraction chunks for qkv
PMB = D // 128     # 8 output-row blocks for c_proj

_cache = {}


def _build():
    import os
    import concourse.bacc as bacc
    import concourse.tile as tile
    import concourse.mybir as mybir

    dt = mybir.dt
    f32, bf16 = dt.float32, dt.bfloat16
    Exp = mybir.ActivationFunctionType.Exp
    if os.environ.get("PROBE_NOEXP"):
        Exp = mybir.ActivationFunctionType.Copy
    mult = mybir.AluOpType.mult

    nc = bacc.Bacc("TRN2", target_bir_lowering=False, debug=False, num_devices=NCORES)

    hidden_t = nc.dram_tensor("hidden_t", [D, S], bf16, kind="ExternalInput")
    w_qkv = nc.dram_tensor("w_qkv", [D, 6 * 128], bf16, kind="ExternalInput")
    b_qkv = nc.dram_tensor("b_qkv", [128, 6], f32, kind="ExternalInput")
    wv = nc.dram_tensor("wv", [HPC, hd, hd], bf16, kind="ExternalInput")
    w_proj = nc.dram_tensor("w_proj", [HPC, hd, D], bf16, kind="ExternalInput")
    maskbig = nc.dram_tensor("maskbig", [128, 896], bf16, kind="ExternalInput")
    out_t = nc.dram_tensor("out_t", [D, S], bf16, kind="ExternalOutput")

    with tile.TileContext(nc) as tc:
        with (
            tc.tile_pool(name="persist", bufs=1) as pp,
            tc.tile_pool(name="work", bufs=4) as wp,
            tc.tile_pool(name="epool", bufs=36) as ep,
            tc.tile_pool(name="ostage", bufs=3) as op,
            tc.tile_pool(name="dscr", bufs=4, space="DRAM") as dr,
            tc.tile_pool(name="ps_big", bufs=int(os.environ.get("PSBUFS", "3")), space="PSUM") as ps_big,
            tc.tile_pool(name="ps_o", bufs=3, space="PSUM") as ps_o,
            tc.tile_pool(name="ps_gen", bufs=2, space="PSUM") as ps_gen,
        ):
            # ---- load weights / hidden ----
            hT = []
            wq = []
            for d in range(DC):
                t = pp.tile([128, S], bf16, tag=f"hT{d}", name=f"hT{d}")
                nc.sync.dma_start(t[:], hidden_t.ap()[d * 128:(d + 1) * 128, :])
                hT.append(t)
                w = pp.tile([128, 6 * 128], bf16, tag=f"wq{d}", name=f"wq{d}")
                nc.sync.dma_start(w[:], w_qkv.ap()[d * 128:(d + 1) * 128, :])
                wq.append(w)
            bias = pp.tile([128, 6], f32, tag="bias", name="bias")
            nc.sync.dma_start(bias[:], b_qkv.ap())
            maskt = pp.tile([128, 896], bf16, tag="mask", name="maskt")
            nc.sync.dma_start(maskt[:], maskbig.ap())

            wv_t, wpj = [], []
            for h in range(HPC):
                p = (h % 2) * 64
                t = pp.tile([128, hd], bf16, tag=f"wv{h}", name=f"wv{h}")
                nc.sync.dma_start(t[p:p + 64, :], wv.ap()[h])
                wv_t.append(t)
            for p in range(2):
                t = pp.tile([128, D], bf16, tag=f"wpj{p}", name=f"wpj{p}")
                nc.sync.dma_start(t[0:hd, :], w_proj.ap()[2 * p])
                nc.sync.dma_start(t[hd:128, :], w_proj.ap()[2 * p + 1])
                wpj.append(t)

            # ---- qkv^T: 6 m-blocks (q|q, k'|k', v|v head pairs) x 4 s-blocks ----
            qq = [pp.tile([128, S], bf16, tag=f"qq{p}", name=f"qq{p}") for p in range(2)]
            kk = [pp.tile([128, S], bf16, tag=f"kk{p}", name=f"kk{p}") for p in range(2)]
            vt = [pp.tile([128, S], bf16, tag=f"vt{p}", name=f"vt{p}") for p in range(2)]
            dests = qq + kk + vt
            for sb in range(NSB):
                for mb in range(6):
                    ps = ps_big.tile([128, SB], f32, tag="psS", name="psS")
                    for d in range(DC):
                        nc.tensor.matmul(
                            ps[:],
                            wq[d][:, mb * 128:(mb + 1) * 128],
                            hT[d][:, sb * SB:(sb + 1) * SB],
                            start=(d == 0),
                            stop=(d == DC - 1),
                        )
                    nc.vector.tensor_scalar_add(
                        out=dests[mb][:, sb * SB:(sb + 1) * SB],
                        in0=ps[:],
                        scalar1=bias[:, mb:mb + 1],
                    )

            def rows(h):
                p = (h % 2) * 64
                return slice(p, p + 64)

            def qT(h):
                return qq[h // 2][rows(h), :]

            def kdecT(h):
                return kk[h // 2][rows(h), :]

            def vT(h):
                return vt[h // 2][rows(h), :]

            # ---- per-head v decompress (W_v folded on host) + ones column ----
            vdo = [pp.tile([128, NKT * (hd + 1)], bf16, tag=f"vdo{h}", name=f"vdo{h}") for h in range(HPC)]
            for h in range(HPC):
                nc.vector.memset(vdo[h][:], 1.0)
                for st in range(NKT):
                    ps = ps_gen.tile([128, hd], f32, tag="psC", name="psC")
                    nc.tensor.matmul(
                        ps[:],
                        vT(h)[:, st * 128:(st + 1) * 128],
                        wv_t[h][rows(h), :],
                    )
                    nc.vector.tensor_copy(
                        vdo[h][:, st * (hd + 1):st * (hd + 1) + hd], ps[:]
                    )

            # ---- attention + merge ----
            # attn packed in head pairs for K=128 c_proj: tile p rows 0-63 =
            # head 2p, rows 64-127 = head 2p+1 (odd heads via DMA shift)
            attn = [pp.tile([128, S], bf16, tag=f"attn{p}", name=f"attn{p}") for p in range(2)]
            for h in range(HPC):
                for qsb in range(NSB):
                    qsl = slice(qsb * SB, (qsb + 1) * SB)
                    nkb = 4 * qsb + 4
                    pso = ps_o.tile([hd + 1, SB], f32, tag="psO", name="psO")
                    for kb in range(nkb):
                        r = kb - 4 * qsb
                        c0 = max(r, 0) * 128  # cols < c0 are causally dead
                        psS = ps_big.tile([128, SB], f32, tag="psS", name="psS")
                        nc.tensor.matmul(
                            psS[:, c0:SB],
                            kdecT(h)[:, kb * 128:(kb + 1) * 128],
                            qT(h)[:, qsb * SB + c0:(qsb + 1) * SB],
                        )
                        e = ep.tile([128, SB], bf16, tag="E", name="e")
                        if r < 0:
                            nc.scalar.activation(e[:], psS[:], Exp)
                        else:
                            # band tile: one diagonal 128-col block, rest valid
                            c1 = c0 + 128
                            et = wp.tile([128, 128], bf16, tag="etd", name="etd")
                            nc.scalar.activation(et[:], psS[:, c0:c1], Exp)
                            nc.vector.tensor_tensor(
                                e[:, c0:c1], et[:], maskt[:, 384:512], mult
                            )
                            if c1 < SB:
                                nc.scalar.activation(e[:, c1:SB], psS[:, c1:SB], Exp)
                        nc.tensor.matmul(
                            pso[:, c0:SB],
                            vdo[h][:, kb * (hd + 1):(kb + 1) * (hd + 1)],
                            e[:, c0:SB],
                            start=(kb == 0),
                            stop=(kb == nkb - 1),
                        )
                    # normalize: num/den via DMA-bounced denominator broadcast
                    nsb = wp.tile([hd + 1, SB], bf16, tag="nsb", name="nsb")
                    nc.vector.tensor_copy(nsb[:], pso[:])
                    den_d = dr.tile([SB], bf16, tag="den_d", name="den_d")
                    nc.sync.dma_start(den_d[:], nsb[hd:hd + 1, :])
                    den_c = wp.tile([128, 4], bf16, tag="den_c", name="den_c")
                    nc.sync.dma_start(
                        den_c[:], den_d[:].rearrange("(p j) -> p j", p=128)
                    )
                    rec_c = wp.tile([128, 4], bf16, tag="rec_c", name="rec_c")
                    with nc.allow_low_precision(reason="softmax denom recip in bf16"):
                        nc.vector.reciprocal(rec_c[:], den_c[:])
                    rec_d = dr.tile([SB], bf16, tag="rec_d", name="rec_d")
                    nc.sync.dma_start(
                        rec_d[:].rearrange("(p j) -> p j", p=128), rec_c[:]
                    )
                    bcast = wp.tile([hd, SB], bf16, tag="bcast", name="bcast")
                    nc.sync.dma_start(
                        bcast[:], rec_d[:].unsqueeze(0).to_broadcast([hd, SB])
                    )
                    if h % 2 == 0:
                        nc.vector.tensor_tensor(
                            attn[h // 2][0:hd, qsl], nsb[0:hd, :], bcast[:], mult
                        )
                    else:
                        # odd head lands on partitions 64-127: DVE can't cross
                        # partitions, so mul into a tmp then DMA-shift
                        atmp = wp.tile([hd, SB], bf16, tag="atmp", name="atmp")
                        nc.vector.tensor_tensor(atmp[:], nsb[0:hd, :], bcast[:], mult)
                        nc.sync.dma_start(attn[h // 2][hd:128, qsl], atmp[:])
                    pace = float(os.environ.get("PACE", "0") or 0)
                    if pace > 0:
                        blk_cyc = sum(
                            (SB - max(kb - 4 * qsb, 0) * 128) * 2 + 120
                            for kb in range(nkb)
                        )
                        nc.tensor.nop(cycle_cnt=int(pace * blk_cyc))

            # ---- partial c_proj: out^T[mb*128:, sb*512:], K=128 per head pair ----
            for sb in range(NSB):
                sl = slice(sb * SB, (sb + 1) * SB)
                for mb in range(PMB):
                    ps = ps_big.tile([128, SB], f32, tag="psS", name="psS")
                    for p in range(2):
                        nc.tensor.matmul(
                            ps[:],
                            wpj[p][:, mb * 128:(mb + 1) * 128],
                            attn[p][:, sl],
                            start=(p == 0),
                            stop=(p == 1),
                        )
                    stage = op.tile([128, SB], bf16, tag="stage", name="stage")
                    nc.vector.tensor_copy(stage[:], ps[:])
                    nc.sync.dma_start(out_t.ap()[mb * 128:(mb + 1) * 128, sl], stage[:])

    nc.compile()
    return nc


def _prep_inputs(hidden_states, w_attn, b_attn, wk_c, wv_c, wk_d, wv_d, w_proj):
    """Per-core input maps (host-side shard + pack + bf16 cast).

    The KV compressor is linear + low-rank, so it folds on host:
      W_k[h] = wk_c[h] @ wk_d[h] / sqrt(hd)  -> folded into w_attn k-columns
      W_v[h] = wv_c[h] @ wv_d[h]             -> single on-device matmul
    """
    hidden_T = [np.ascontiguousarray(hidden_states[b].T).astype(BF16) for b in range(B)]
    Wk = np.einsum("hdc,hce->hde", wk_c.astype(np.float64),
                   wk_d.astype(np.float64)) * (1.0 / np.sqrt(hd))  # [H,hd,hd]
    Wv = np.einsum("hdc,hce->hde", wv_c.astype(np.float64),
                   wv_d.astype(np.float64))                        # [H,hd,hd]
    wq_h = lambda h: w_attn[:, h * hd:(h + 1) * hd]
    wk_h = lambda h: (w_attn[:, D + h * hd:D + (h + 1) * hd].astype(np.float64)
                      @ Wk[h]).astype(np.float32)
    wv_h = lambda h: w_attn[:, 2 * D + h * hd:2 * D + (h + 1) * hd]
    bq_h = lambda h: b_attn[h * hd:(h + 1) * hd]
    bk_h = lambda h: (b_attn[D + h * hd:D + (h + 1) * hd].astype(np.float64)
                      @ Wk[h]).astype(np.float32)
    bv_h = lambda h: b_attn[2 * D + h * hd:2 * D + (h + 1) * hd]
    in_maps = []
    for c in range(NCORES):
        b = c // 4
        hs = list(range((c % 4) * HPC, (c % 4) * HPC + HPC))
        # m-blocks: [q0|q1], [q2|q3], [k'0|k'1], [k'2|k'3], [v0|v1], [v2|v3]
        cols, bcols = [], []
        for fn, bfn in ((wq_h, bq_h), (wk_h, bk_h), (wv_h, bv_h)):
            for h in hs:
                cols.append(fn(h))
                bcols.append(bfn(h))
        w_qkv_l = np.concatenate(cols, axis=1).astype(BF16)        # [1024, 768]
        b_qkv_l = (
            np.concatenate(bcols).astype(np.float32).reshape(6, 128).T.copy()
        )                                                          # [128, 6]
        k = np.arange(128).reshape(128, 1)
        cgrid = np.arange(896).reshape(1, 896)
        mask = (k <= cgrid - 384).astype(BF16)
        in_maps.append(
            {
                "hidden_t": hidden_T[b],
                "w_qkv": w_qkv_l,
                "b_qkv": b_qkv_l,
                "wv": Wv[hs].astype(BF16),
                "w_proj": np.stack(
                    [w_proj[h * hd:(h + 1) * hd, :] for h in hs]
                ).astype(BF16),
                "maskbig": np.ascontiguousarray(mask),
            }
        )
    return in_maps


def kernel(
    hidden_states,
    w_attn,
    b_attn,
    w_proj,
    b_proj,
    wk_c,
    wv_c,
    wk_d,
    wv_d,
    _trace=False,
):
    from concourse.bass_utils import run_bass_kernel_spmd

    if "nc" not in _cache:
        _cache["nc"] = _build()
    nc = _cache["nc"]

    in_maps = _prep_inputs(
        np.asarray(hidden_states),
        np.asarray(w_attn),
        np.asarray(b_attn),
        np.asarray(wk_c),
        np.asarray(wv_c),
        np.asarray(wk_d),
        np.asarray(wv_d),
        np.asarray(w_proj),
    )
    res = run_bass_kernel_spmd(
        nc, in_maps, core_ids=list(range(NCORES)), trace=_trace
    )
    out = np.empty((B, S, D), np.float32)
    for b in range(B):
        acc = np.zeros((D, S), np.float32)
        for c in range(4 * b, 4 * b + 4):
            acc += res.results[c]["out_t"].astype(np.float32)
        out[b] = acc.T + np.asarray(b_proj, np.float32)
    if _trace:
        _cache["last_exec_time_ns"] = res.exec_time_ns
        _cache["last_results"] = res
    return out


# revision 20
# speedup vs baseline: 1.6683x; 1.6683x over previous
"""Compressed-KV GPT-2 attention block on 8 TRN2 NeuronCores.

Sharding: batch x head-group. Core c: batch b = c//4, heads 4*(c%4)..4*(c%4)+4.
Each core runs the full fused pipeline for its 4 heads in transposed-activation
layout ([dim, seq] on partitions) and emits a partial c_proj output^T; the host
sums the 4 partials per batch and adds b_proj.

Device pipeline per core (all matmuls bf16 -> fp32 PSUM):
  The KV compressor is low-rank and linear, so host folds it:
    k_dec = k @ (wk_c@wk_d)  -> fold W_k into w_attn k-columns (w_k' = w_k W_k)
    v_dec = v @ (wv_c@wv_d)  -> one small on-device matmul with W_v
  qkv^T   = w_qkv^T-chunks @ hidden^T   (m-blocks: q|q, k'|k', v|v head pairs,
            so kdec^T comes straight out of the qkv matmul)
  vdec    = v^T-slices^T @ W_v          (natural [s,d] + ones col for denom)
  S^T     = kdec^T-slices^T @ q^T   -> exp (no-max softmax; causal via mask mul)
  attn^T  = vdec_ones^T @ E (accum over key tiles; row 64 = softmax denom)
  out^T  += w_proj-rows^T @ attn^T  (partial over this core's heads)
"""

import sys

if "/opt/trn_rl_repo" not in sys.path:
    sys.path.insert(0, "/opt/trn_rl_repo")

import numpy as np
import ml_dtypes

BF16 = ml_dtypes.bfloat16

B, S, D = 2, 2048, 1024
H, hd, C = 16, 64, 32
NCORES = 8
HPC = 4            # heads per core
SB = 512           # free-dim block (PSUM bank / max moving cols)
NSB = S // SB      # 4 seq blocks of 512
NKT = S // 128     # 16 key tiles of 128
DC = D // 128      # 8 contraction chunks for qkv
PMB = D // 128     # 8 output-row blocks for c_proj

_cache = {}


def _build():
    import os
    import concourse.bacc as bacc
    import concourse.tile as tile
    import concourse.mybir as mybir

    dt = mybir.dt
    f32, bf16 = dt.float32, dt.bfloat16
    Exp = mybir.ActivationFunctionType.Exp
    if os.environ.get("PROBE_NOEXP"):
        Exp = mybir.ActivationFunctionType.Copy
    mult = mybir.AluOpType.mult

    nc = bacc.Bacc("TRN2", target_bir_lowering=False, debug=False, num_devices=NCORES)

    hidden_t = nc.dram_tensor("hidden_t", [D, S], bf16, kind="ExternalInput")
    w_qkv = nc.dram_tensor("w_qkv", [D, 6 * 128], bf16, kind="ExternalInput")
    b_qkv = nc.dram_tensor("b_qkv", [128, 6], f32, kind="ExternalInput")
    wv = nc.dram_tensor("wv", [HPC, hd, hd], bf16, kind="ExternalInput")
    w_proj = nc.dram_tensor("w_proj", [HPC, hd, D], bf16, kind="ExternalInput")
    maskbig = nc.dram_tensor("maskbig", [128, 896], bf16, kind="ExternalInput")
    out_t = nc.dram_tensor("out_t", [D, S], bf16, kind="ExternalOutput")

    with tile.TileContext(nc) as tc:
        with (
            tc.tile_pool(name="persist", bufs=1) as pp,
            tc.tile_pool(name="work", bufs=4) as wp,
            tc.tile_pool(name="epool", bufs=36) as ep,
            tc.tile_pool(name="ostage", bufs=3) as op,
            tc.tile_pool(name="dscr", bufs=4, space="DRAM") as dr,
            tc.tile_pool(name="ps_big", bufs=4, space="PSUM") as ps_big,
            tc.tile_pool(name="ps_o", bufs=4, space="PSUM") as ps_o,
        ):
            # ---- load weights / hidden ----
            hT = []
            wq = []
            for d in range(DC):
                t = pp.tile([128, S], bf16, tag=f"hT{d}", name=f"hT{d}")
                nc.sync.dma_start(t[:], hidden_t.ap()[d * 128:(d + 1) * 128, :])
                hT.append(t)
                w = pp.tile([128, 6 * 128], bf16, tag=f"wq{d}", name=f"wq{d}")
                nc.sync.dma_start(w[:], w_qkv.ap()[d * 128:(d + 1) * 128, :])
                wq.append(w)
            bias = pp.tile([128, 6], f32, tag="bias", name="bias")
            nc.sync.dma_start(bias[:], b_qkv.ap())
            maskt = pp.tile([128, 896], bf16, tag="mask", name="maskt")
            nc.sync.dma_start(maskt[:], maskbig.ap())

            wv_t, wpj = [], []
            for h in range(HPC):
                p = (h % 2) * 64
                t = pp.tile([128, hd], bf16, tag=f"wv{h}", name=f"wv{h}")
                nc.sync.dma_start(t[p:p + 64, :], wv.ap()[h])
                wv_t.append(t)
            for p in range(2):
                t = pp.tile([128, D], bf16, tag=f"wpj{p}", name=f"wpj{p}")
                nc.sync.dma_start(t[0:hd, :], w_proj.ap()[2 * p])
                nc.sync.dma_start(t[hd:128, :], w_proj.ap()[2 * p + 1])
                wpj.append(t)

            # ---- qkv^T: 6 m-blocks (q|q, k'|k', v|v head pairs) x 4 s-blocks ----
            qq = [pp.tile([128, S], bf16, tag=f"qq{p}", name=f"qq{p}") for p in range(2)]
            kk = [pp.tile([128, S], bf16, tag=f"kk{p}", name=f"kk{p}") for p in range(2)]
            vt = [pp.tile([128, S], bf16, tag=f"vt{p}", name=f"vt{p}") for p in range(2)]
            dests = qq + kk + vt
            for sb in range(NSB):
                for mb in range(6):
                    ps = ps_big.tile([128, SB], f32, tag="psS", name="psS")
                    for d in range(DC):
                        nc.tensor.matmul(
                            ps[:],
                            wq[d][:, mb * 128:(mb + 1) * 128],
                            hT[d][:, sb * SB:(sb + 1) * SB],
                            start=(d == 0),
                            stop=(d == DC - 1),
                        )
                    nc.vector.tensor_scalar_add(
                        out=dests[mb][:, sb * SB:(sb + 1) * SB],
                        in0=ps[:],
                        scalar1=bias[:, mb:mb + 1],
                    )

            def rows(h):
                p = (h % 2) * 64
                return slice(p, p + 64)

            def qT(h):
                return qq[h // 2][rows(h), :]

            def kdecT(h):
                return kk[h // 2][rows(h), :]

            def vT(h):
                return vt[h // 2][rows(h), :]

            # ---- per-head v decompress (W_v folded on host) + ones column ----
            vdo = [pp.tile([128, NKT * (hd + 1)], bf16, tag=f"vdo{h}", name=f"vdo{h}") for h in range(HPC)]
            for h in range(HPC):
                nc.vector.memset(vdo[h][:], 1.0)
                for st in range(NKT):
                    ps = ps_o.tile([128, hd], f32, tag="psO", name="psC")
                    nc.tensor.matmul(
                        ps[:],
                        vT(h)[:, st * 128:(st + 1) * 128],
                        wv_t[h][rows(h), :],
                    )
                    nc.vector.tensor_copy(
                        vdo[h][:, st * (hd + 1):st * (hd + 1) + hd], ps[:]
                    )

            # ---- attention + merge ----
            # attn packed in head pairs for K=128 c_proj: tile p rows 0-63 =
            # head 2p, rows 64-127 = head 2p+1 (odd heads via DMA shift)
            attn = [pp.tile([128, S], bf16, tag=f"attn{p}", name=f"attn{p}") for p in range(2)]
            for qsb in range(NSB):
                for h in range(HPC):
                    qsl = slice(qsb * SB, (qsb + 1) * SB)
                    nkb = 4 * qsb + 4
                    pso = ps_o.tile([hd + 1, SB], f32, tag="psO", name="psO")
                    for kb in range(nkb):
                        r = kb - 4 * qsb
                        c0 = max(r, 0) * 128  # cols < c0 are causally dead
                        psS = ps_big.tile([128, SB], f32, tag="psS", name="psS")
                        nc.tensor.matmul(
                            psS[:, c0:SB],
                            kdecT(h)[:, kb * 128:(kb + 1) * 128],
                            qT(h)[:, qsb * SB + c0:(qsb + 1) * SB],
                        )
                        e = ep.tile([128, SB], bf16, tag="E", name="e")
                        if r < 0:
                            nc.scalar.activation(e[:], psS[:], Exp)
                        else:
                            # band tile: one diagonal 128-col block, rest valid
                            c1 = c0 + 128
                            et = wp.tile([128, 128], bf16, tag="etd", name="etd")
                            nc.scalar.activation(et[:], psS[:, c0:c1], Exp)
                            nc.vector.tensor_tensor(
                                e[:, c0:c1], et[:], maskt[:, 384:512], mult
                            )
                            if c1 < SB:
                                nc.scalar.activation(e[:, c1:SB], psS[:, c1:SB], Exp)
                        nc.tensor.matmul(
                            pso[:, c0:SB],
                            vdo[h][:, kb * (hd + 1):(kb + 1) * (hd + 1)],
                            e[:, c0:SB],
                            start=(kb == 0),
                            stop=(kb == nkb - 1),
                        )
                    # normalize: num/den via DMA-bounced denominator broadcast
                    nsb = wp.tile([hd + 1, SB], bf16, tag="nsb", name="nsb")
                    nc.vector.tensor_copy(nsb[:], pso[:])
                    den_d = dr.tile([SB], bf16, tag="den_d", name="den_d")
                    nc.sync.dma_start(den_d[:], nsb[hd:hd + 1, :])
                    den_c = wp.tile([128, 4], bf16, tag="den_c", name="den_c")
                    nc.sync.dma_start(
                        den_c[:], den_d[:].rearrange("(p j) -> p j", p=128)
                    )
                    rec_c = wp.tile([128, 4], bf16, tag="rec_c", name="rec_c")
                    with nc.allow_low_precision(reason="softmax denom recip in bf16"):
                        nc.vector.reciprocal(rec_c[:], den_c[:])
                    rec_d = dr.tile([SB], bf16, tag="rec_d", name="rec_d")
                    nc.sync.dma_start(
                        rec_d[:].rearrange("(p j) -> p j", p=128), rec_c[:]
                    )
                    bcast = wp.tile([hd, SB], bf16, tag="bcast", name="bcast")
                    nc.sync.dma_start(
                        bcast[:], rec_d[:].unsqueeze(0).to_broadcast([hd, SB])
                    )
                    if h % 2 == 0:
                        nc.vector.tensor_tensor(
                            attn[h // 2][0:hd, qsl], nsb[0:hd, :], bcast[:], mult
                        )
                    else:
                        # odd head lands on partitions 64-127: DVE can't cross
                        # partitions, so mul into a tmp then DMA-shift
                        atmp = wp.tile([hd, SB], bf16, tag="atmp", name="atmp")
                        nc.vector.tensor_tensor(atmp[:], nsb[0:hd, :], bcast[:], mult)
                        nc.sync.dma_start(attn[h // 2][hd:128, qsl], atmp[:])
                    pace = float(os.environ.get("PACE", "0") or 0)
                    if pace > 0:
                        blk_cyc = sum(
                            (SB - max(kb - 4 * qsb, 0) * 128) * 2 + 120
                            for kb in range(nkb)
                        )
                        nc.tensor.nop(cycle_cnt=int(pace * blk_cyc))

                # partial c_proj for this s-block: out^T[mb*128:, qsb*512:]
                for mb in range(PMB):
                    ps = ps_big.tile([128, SB], f32, tag="psS", name="psP")
                    for p in range(2):
                        nc.tensor.matmul(
                            ps[:],
                            wpj[p][:, mb * 128:(mb + 1) * 128],
                            attn[p][:, qsl],
                            start=(p == 0),
                            stop=(p == 1),
                        )
                    stage = op.tile([128, SB], bf16, tag="stage", name="stage")
                    nc.vector.tensor_copy(stage[:], ps[:])
                    nc.sync.dma_start(
                        out_t.ap()[mb * 128:(mb + 1) * 128, qsl], stage[:]
                    )

    nc.compile()
    return nc


def _prep_inputs(hidden_states, w_attn, b_attn, wk_c, wv_c, wk_d, wv_d, w_proj):
    """Per-core input maps (host-side shard + pack + bf16 cast).

    The KV compressor is linear + low-rank, so it folds on host:
      W_k[h] = wk_c[h] @ wk_d[h] / sqrt(hd)  -> folded into w_attn k-columns
      W_v[h] = wv_c[h] @ wv_d[h]             -> single on-device matmul
    """
    hidden_T = [np.ascontiguousarray(hidden_states[b].T).astype(BF16) for b in range(B)]
    Wk = np.einsum("hdc,hce->hde", wk_c.astype(np.float64),
                   wk_d.astype(np.float64)) * (1.0 / np.sqrt(hd))  # [H,hd,hd]
    Wv = np.einsum("hdc,hce->hde", wv_c.astype(np.float64),
                   wv_d.astype(np.float64))                        # [H,hd,hd]
    wq_h = lambda h: w_attn[:, h * hd:(h + 1) * hd]
    wk_h = lambda h: (w_attn[:, D + h * hd:D + (h + 1) * hd].astype(np.float64)
                      @ Wk[h]).astype(np.float32)
    wv_h = lambda h: w_attn[:, 2 * D + h * hd:2 * D + (h + 1) * hd]
    bq_h = lambda h: b_attn[h * hd:(h + 1) * hd]
    bk_h = lambda h: (b_attn[D + h * hd:D + (h + 1) * hd].astype(np.float64)
                      @ Wk[h]).astype(np.float32)
    bv_h = lambda h: b_attn[2 * D + h * hd:2 * D + (h + 1) * hd]
    in_maps = []
    for c in range(NCORES):
        b = c // 4
        hs = list(range((c % 4) * HPC, (c % 4) * HPC + HPC))
        # m-blocks: [q0|q1], [q2|q3], [k'0|k'1], [k'2|k'3], [v0|v1], [v2|v3]
        cols, bcols = [], []
        for fn, bfn in ((wq_h, bq_h), (wk_h, bk_h), (wv_h, bv_h)):
            for h in hs:
                cols.append(fn(h))
                bcols.append(bfn(h))
        w_qkv_l = np.concatenate(cols, axis=1).astype(BF16)        # [1024, 768]
        b_qkv_l = (
            np.concatenate(bcols).astype(np.float32).reshape(6, 128).T.copy()
        )                                                          # [128, 6]
        k = np.arange(128).reshape(128, 1)
        cgrid = np.arange(896).reshape(1, 896)
        mask = (k <= cgrid - 384).astype(BF16)
        in_maps.append(
            {
                "hidden_t": hidden_T[b],
                "w_qkv": w_qkv_l,
                "b_qkv": b_qkv_l,
                "wv": Wv[hs].astype(BF16),
                "w_proj": np.stack(
                    [w_proj[h * hd:(h + 1) * hd, :] for h in hs]
                ).astype(BF16),
                "maskbig": np.ascontiguousarray(mask),
            }
        )
    return in_maps


def kernel(
    hidden_states,
    w_attn,
    b_attn,
    w_proj,
    b_proj,
    wk_c,
    wv_c,
    wk_d,
    wv_d,
    _trace=False,
):
    from concourse.bass_utils import run_bass_kernel_spmd

    if "nc" not in _cache:
        _cache["nc"] = _build()
    nc = _cache["nc"]

    in_maps = _prep_inputs(
        np.asarray(hidden_states),
        np.asarray(w_attn),
        np.asarray(b_attn),
        np.asarray(wk_c),
        np.asarray(wv_c),
        np.asarray(wk_d),
        np.asarray(wv_d),
        np.asarray(w_proj),
    )
    res = run_bass_kernel_spmd(
        nc, in_maps, core_ids=list(range(NCORES)), trace=_trace
    )
    out = np.empty((B, S, D), np.float32)
    for b in range(B):
        acc = np.zeros((D, S), np.float32)
        for c in range(4 * b, 4 * b + 4):
            acc += res.results[c]["out_t"].astype(np.float32)
        out[b] = acc.T + np.asarray(b_proj, np.float32)
    if _trace:
        _cache["last_exec_time_ns"] = res.exec_time_ns
        _cache["last_results"] = res
    return out


# revision 21
# speedup vs baseline: 1.6891x; 1.0124x over previous
"""Compressed-KV GPT-2 attention block on 8 TRN2 NeuronCores.

Sharding: batch x head-group. Core c: batch b = c//4, heads 4*(c%4)..4*(c%4)+4.
Each core runs the full fused pipeline for its 4 heads in transposed-activation
layout ([dim, seq] on partitions) and emits a partial c_proj output^T; the host
sums the 4 partials per batch and adds b_proj.

Device pipeline per core (all matmuls bf16 -> fp32 PSUM):
  The KV compressor is low-rank and linear, so host folds it:
    k_dec = k @ (wk_c@wk_d)  -> fold W_k into w_attn k-columns (w_k' = w_k W_k)
    v_dec = v @ (wv_c@wv_d)  -> one small on-device matmul with W_v
  qkv^T   = w_qkv^T-chunks @ hidden^T   (m-blocks: q|q, k'|k', v|v head pairs,
            so kdec^T comes straight out of the qkv matmul)
  vdec    = v^T-slices^T @ W_v          (natural [s,d] + ones col for denom)
  S^T     = kdec^T-slices^T @ q^T   -> exp (no-max softmax; causal via mask mul)
  attn^T  = vdec_ones^T @ E (accum over key tiles; row 64 = softmax denom)
  out^T  += w_proj-rows^T @ attn^T  (partial over this core's heads)
"""

import sys

if "/opt/trn_rl_repo" not in sys.path:
    sys.path.insert(0, "/opt/trn_rl_repo")

import numpy as np
import ml_dtypes

BF16 = ml_dtypes.bfloat16

B, S, D = 2, 2048, 1024
H, hd, C = 16, 64, 32
NCORES = 8
HPC = 4            # heads per core
SB = 512           # free-dim block (PSUM bank / max moving cols)
NSB = S // SB      # 4 seq blocks of 512
NKT = S // 128     # 16 key tiles of 128
DC = D // 128      # 8 contraction chunks for qkv
PMB = D // 128     # 8 output-row blocks for c_proj

_cache = {}


def _build():
    import os
    import concourse.bacc as bacc
    import concourse.tile as tile
    import concourse.mybir as mybir

    dt = mybir.dt
    f32, bf16 = dt.float32, dt.bfloat16
    Exp = mybir.ActivationFunctionType.Exp
    if os.environ.get("PROBE_NOEXP"):
        Exp = mybir.ActivationFunctionType.Copy
    mult = mybir.AluOpType.mult

    nc = bacc.Bacc("TRN2", target_bir_lowering=False, debug=False, num_devices=NCORES)

    hidden_t = nc.dram_tensor("hidden_t", [D, S], bf16, kind="ExternalInput")
    w_qkv = nc.dram_tensor("w_qkv", [D, 6 * 128], bf16, kind="ExternalInput")
    b_qkv = nc.dram_tensor("b_qkv", [128, 6], f32, kind="ExternalInput")
    wv = nc.dram_tensor("wv", [HPC, hd, hd], bf16, kind="ExternalInput")
    w_proj = nc.dram_tensor("w_proj", [HPC, hd, D], bf16, kind="ExternalInput")
    maskbig = nc.dram_tensor("maskbig", [128, 896], bf16, kind="ExternalInput")
    out_t = nc.dram_tensor("out_t", [D, S], bf16, kind="ExternalOutput")

    with tile.TileContext(nc) as tc:
        with (
            tc.tile_pool(name="persist", bufs=1) as pp,
            tc.tile_pool(name="work", bufs=4) as wp,
            tc.tile_pool(name="epool", bufs=36) as ep,
            tc.tile_pool(name="ostage", bufs=3) as op,
            tc.tile_pool(name="dscr", bufs=4, space="DRAM") as dr,
            tc.tile_pool(name="ps_big", bufs=4, space="PSUM") as ps_big,
            tc.tile_pool(name="ps_o", bufs=4, space="PSUM") as ps_o,
        ):
            # ---- load weights / hidden ----
            hT = []
            wq = []
            for d in range(DC):
                t = pp.tile([128, S], bf16, tag=f"hT{d}", name=f"hT{d}")
                nc.sync.dma_start(t[:], hidden_t.ap()[d * 128:(d + 1) * 128, :])
                hT.append(t)
                w = pp.tile([128, 6 * 128], bf16, tag=f"wq{d}", name=f"wq{d}")
                nc.sync.dma_start(w[:], w_qkv.ap()[d * 128:(d + 1) * 128, :])
                wq.append(w)
            bias = pp.tile([128, 6], f32, tag="bias", name="bias")
            nc.sync.dma_start(bias[:], b_qkv.ap())
            maskt = pp.tile([128, 896], bf16, tag="mask", name="maskt")
            nc.sync.dma_start(maskt[:], maskbig.ap())

            wv_t, wpj = [], []
            for h in range(HPC):
                p = (h % 2) * 64
                t = pp.tile([128, hd], bf16, tag=f"wv{h}", name=f"wv{h}")
                nc.sync.dma_start(t[p:p + 64, :], wv.ap()[h])
                wv_t.append(t)
            for p in range(2):
                t = pp.tile([128, D], bf16, tag=f"wpj{p}", name=f"wpj{p}")
                nc.sync.dma_start(t[0:hd, :], w_proj.ap()[2 * p])
                nc.sync.dma_start(t[hd:128, :], w_proj.ap()[2 * p + 1])
                wpj.append(t)

            # ---- qkv^T: 6 m-blocks (q|q, k'|k', v|v head pairs) x 4 s-blocks ----
            qq = [pp.tile([128, S], bf16, tag=f"qq{p}", name=f"qq{p}") for p in range(2)]
            kk = [pp.tile([128, S], bf16, tag=f"kk{p}", name=f"kk{p}") for p in range(2)]
            vt = [pp.tile([128, S], bf16, tag=f"vt{p}", name=f"vt{p}") for p in range(2)]
            dests = qq + kk + vt
            for sb in range(NSB):
                for mb in range(6):
                    ps = ps_big.tile([128, SB], f32, tag="psS", name="psS")
                    for d in range(DC):
                        nc.tensor.matmul(
                            ps[:],
                            wq[d][:, mb * 128:(mb + 1) * 128],
                            hT[d][:, sb * SB:(sb + 1) * SB],
                            start=(d == 0),
                            stop=(d == DC - 1),
                        )
                    nc.vector.tensor_scalar_add(
                        out=dests[mb][:, sb * SB:(sb + 1) * SB],
                        in0=ps[:],
                        scalar1=bias[:, mb:mb + 1],
                    )

            def rows(h):
                p = (h % 2) * 64
                return slice(p, p + 64)

            def qT(h):
                return qq[h // 2][rows(h), :]

            def kdecT(h):
                return kk[h // 2][rows(h), :]

            def vT(h):
                return vt[h // 2][rows(h), :]

            # ---- per-head v decompress (W_v folded on host) + ones column ----
            vdo = [pp.tile([128, NKT * (hd + 1)], bf16, tag=f"vdo{h}", name=f"vdo{h}") for h in range(HPC)]
            for h in range(HPC):
                nc.vector.memset(vdo[h][:], 1.0)
                for st in range(NKT):
                    ps = ps_o.tile([128, hd], f32, tag="psO", name="psC")
                    nc.tensor.matmul(
                        ps[:],
                        vT(h)[:, st * 128:(st + 1) * 128],
                        wv_t[h][rows(h), :],
                    )
                    nc.vector.tensor_copy(
                        vdo[h][:, st * (hd + 1):st * (hd + 1) + hd], ps[:]
                    )

            # ---- attention + merge ----
            # attn packed in head pairs for K=128 c_proj: tile p rows 0-63 =
            # head 2p, rows 64-127 = head 2p+1 (odd heads via DMA shift)
            attn = [pp.tile([128, S], bf16, tag=f"attn{p}", name=f"attn{p}") for p in range(2)]
            for h in range(HPC):
                for qsb in range(NSB):
                    qsl = slice(qsb * SB, (qsb + 1) * SB)
                    nkb = 4 * qsb + 4
                    pso = ps_o.tile([hd + 1, SB], f32, tag="psO", name="psO")
                    for kb in range(nkb):
                        r = kb - 4 * qsb
                        c0 = max(r, 0) * 128  # cols < c0 are causally dead
                        psS = ps_big.tile([128, SB], f32, tag="psS", name="psS")
                        nc.tensor.matmul(
                            psS[:, c0:SB],
                            kdecT(h)[:, kb * 128:(kb + 1) * 128],
                            qT(h)[:, qsb * SB + c0:(qsb + 1) * SB],
                        )
                        e = ep.tile([128, SB], bf16, tag="E", name="e")
                        if r < 0:
                            nc.scalar.activation(e[:], psS[:], Exp)
                        else:
                            # band tile: one diagonal 128-col block, rest valid
                            c1 = c0 + 128
                            et = wp.tile([128, 128], bf16, tag="etd", name="etd")
                            nc.scalar.activation(et[:], psS[:, c0:c1], Exp)
                            nc.vector.tensor_tensor(
                                e[:, c0:c1], et[:], maskt[:, 384:512], mult
                            )
                            if c1 < SB:
                                nc.scalar.activation(e[:, c1:SB], psS[:, c1:SB], Exp)
                        nc.tensor.matmul(
                            pso[:, c0:SB],
                            vdo[h][:, kb * (hd + 1):(kb + 1) * (hd + 1)],
                            e[:, c0:SB],
                            start=(kb == 0),
                            stop=(kb == nkb - 1),
                        )
                    # normalize: num/den via DMA-bounced denominator broadcast
                    nsb = wp.tile([hd + 1, SB], bf16, tag="nsb", name="nsb")
                    nc.vector.tensor_copy(nsb[:], pso[:])
                    den_d = dr.tile([SB], bf16, tag="den_d", name="den_d")
                    nc.sync.dma_start(den_d[:], nsb[hd:hd + 1, :])
                    den_c = wp.tile([128, 4], bf16, tag="den_c", name="den_c")
                    nc.sync.dma_start(
                        den_c[:], den_d[:].rearrange("(p j) -> p j", p=128)
                    )
                    rec_c = wp.tile([128, 4], bf16, tag="rec_c", name="rec_c")
                    with nc.allow_low_precision(reason="softmax denom recip in bf16"):
                        nc.vector.reciprocal(rec_c[:], den_c[:])
                    rec_d = dr.tile([SB], bf16, tag="rec_d", name="rec_d")
                    nc.sync.dma_start(
                        rec_d[:].rearrange("(p j) -> p j", p=128), rec_c[:]
                    )
                    bcast = wp.tile([hd, SB], bf16, tag="bcast", name="bcast")
                    nc.sync.dma_start(
                        bcast[:], rec_d[:].unsqueeze(0).to_broadcast([hd, SB])
                    )
                    if h % 2 == 0:
                        nc.vector.tensor_tensor(
                            attn[h // 2][0:hd, qsl], nsb[0:hd, :], bcast[:], mult
                        )
                    else:
                        # odd head lands on partitions 64-127: DVE can't cross
                        # partitions, so mul into a tmp then DMA-shift
                        atmp = wp.tile([hd, SB], bf16, tag="atmp", name="atmp")
                        nc.vector.tensor_tensor(atmp[:], nsb[0:hd, :], bcast[:], mult)
                        nc.sync.dma_start(attn[h // 2][hd:128, qsl], atmp[:])
                    pace = float(os.environ.get("PACE", "0") or 0)
                    if pace > 0:
                        blk_cyc = sum(
                            (SB - max(kb - 4 * qsb, 0) * 128) * 2 + 120
                            for kb in range(nkb)
                        )
                        nc.tensor.nop(cycle_cnt=int(pace * blk_cyc))

            # ---- partial c_proj: out^T[mb*128:, sb*512:], K=128 per pair ----
            for sb in range(NSB):
                sl = slice(sb * SB, (sb + 1) * SB)
                for mb in range(PMB):
                    ps = ps_big.tile([128, SB], f32, tag="psS", name="psP")
                    for p in range(2):
                        nc.tensor.matmul(
                            ps[:],
                            wpj[p][:, mb * 128:(mb + 1) * 128],
                            attn[p][:, sl],
                            start=(p == 0),
                            stop=(p == 1),
                        )
                    stage = op.tile([128, SB], bf16, tag="stage", name="stage")
                    nc.vector.tensor_copy(stage[:], ps[:])
                    nc.sync.dma_start(out_t.ap()[mb * 128:(mb + 1) * 128, sl], stage[:])

    nc.compile()
    return nc


def _prep_inputs(hidden_states, w_attn, b_attn, wk_c, wv_c, wk_d, wv_d, w_proj):
    """Per-core input maps (host-side shard + pack + bf16 cast).

    The KV compressor is linear + low-rank, so it folds on host:
      W_k[h] = wk_c[h] @ wk_d[h] / sqrt(hd)  -> folded into w_attn k-columns
      W_v[h] = wv_c[h] @ wv_d[h]             -> single on-device matmul
    """
    hidden_T = [np.ascontiguousarray(hidden_states[b].T).astype(BF16) for b in range(B)]
    Wk = np.einsum("hdc,hce->hde", wk_c.astype(np.float64),
                   wk_d.astype(np.float64)) * (1.0 / np.sqrt(hd))  # [H,hd,hd]
    Wv = np.einsum("hdc,hce->hde", wv_c.astype(np.float64),
                   wv_d.astype(np.float64))                        # [H,hd,hd]
    wq_h = lambda h: w_attn[:, h * hd:(h + 1) * hd]
    wk_h = lambda h: (w_attn[:, D + h * hd:D + (h + 1) * hd].astype(np.float64)
                      @ Wk[h]).astype(np.float32)
    wv_h = lambda h: w_attn[:, 2 * D + h * hd:2 * D + (h + 1) * hd]
    bq_h = lambda h: b_attn[h * hd:(h + 1) * hd]
    bk_h = lambda h: (b_attn[D + h * hd:D + (h + 1) * hd].astype(np.float64)
                      @ Wk[h]).astype(np.float32)
    bv_h = lambda h: b_attn[2 * D + h * hd:2 * D + (h + 1) * hd]
    in_maps = []
    for c in range(NCORES):
        b = c // 4
        hs = list(range((c % 4) * HPC, (c % 4) * HPC + HPC))
        # m-blocks: [q0|q1], [q2|q3], [k'0|k'1], [k'2|k'3], [v0|v1], [v2|v3]
        cols, bcols = [], []
        for fn, bfn in ((wq_h, bq_h), (wk_h, bk_h), (wv_h, bv_h)):
            for h in hs:
                cols.append(fn(h))
                bcols.append(bfn(h))
        w_qkv_l = np.concatenate(cols, axis=1).astype(BF16)        # [1024, 768]
        b_qkv_l = (
            np.concatenate(bcols).astype(np.float32).reshape(6, 128).T.copy()
        )                                                          # [128, 6]
        k = np.arange(128).reshape(128, 1)
        cgrid = np.arange(896).reshape(1, 896)
        mask = (k <= cgrid - 384).astype(BF16)
        in_maps.append(
            {
                "hidden_t": hidden_T[b],
                "w_qkv": w_qkv_l,
                "b_qkv": b_qkv_l,
                "wv": Wv[hs].astype(BF16),
                "w_proj": np.stack(
                    [w_proj[h * hd:(h + 1) * hd, :] for h in hs]
                ).astype(BF16),
                "maskbig": np.ascontiguousarray(mask),
            }
        )
    return in_maps


def kernel(
    hidden_states,
    w_attn,
    b_attn,
    w_proj,
    b_proj,
    wk_c,
    wv_c,
    wk_d,
    wv_d,
    _trace=False,
):
    from concourse.bass_utils import run_bass_kernel_spmd

    if "nc" not in _cache:
        _cache["nc"] = _build()
    nc = _cache["nc"]

    in_maps = _prep_inputs(
        np.asarray(hidden_states),
        np.asarray(w_attn),
        np.asarray(b_attn),
        np.asarray(wk_c),
        np.asarray(wv_c),
        np.asarray(wk_d),
        np.asarray(wv_d),
        np.asarray(w_proj),
    )
    res = run_bass_kernel_spmd(
        nc, in_maps, core_ids=list(range(NCORES)), trace=_trace
    )
    out = np.empty((B, S, D), np.float32)
    for b in range(B):
        acc = np.zeros((D, S), np.float32)
        for c in range(4 * b, 4 * b + 4):
            acc += res.results[c]["out_t"].astype(np.float32)
        out[b] = acc.T + np.asarray(b_proj, np.float32)
    if _trace:
        _cache["last_exec_time_ns"] = res.exec_time_ns
        _cache["last_results"] = res
    return out


# revision 22
# speedup vs baseline: 1.7570x; 1.0402x over previous
"""Compressed-KV GPT-2 attention block on 8 TRN2 NeuronCores.

Sharding: batch x head-group. Core c: batch b = c//4, heads 4*(c%4)..4*(c%4)+4.
Each core runs the full fused pipeline for its 4 heads in transposed-activation
layout ([dim, seq] on partitions) and emits a partial c_proj output^T; the host
sums the 4 partials per batch and adds b_proj.

Device pipeline per core (all matmuls bf16 -> fp32 PSUM):
  The KV compressor is low-rank and linear, so host folds it:
    k_dec = k @ (wk_c@wk_d)  -> fold W_k into w_attn k-columns (w_k' = w_k W_k)
    v_dec = v @ (wv_c@wv_d)  -> one small on-device matmul with W_v
  qkv^T   = w_qkv^T-chunks @ hidden^T   (m-blocks: q|q, k'|k', v|v head pairs,
            so kdec^T comes straight out of the qkv matmul)
  vdec    = v^T-slices^T @ W_v          (natural [s,d] + ones col for denom)
  S^T     = kdec^T-slices^T @ q^T   -> exp (no-max softmax; causal via mask mul)
  attn^T  = vdec_ones^T @ E (accum over key tiles; row 64 = softmax denom)
  out^T  += w_proj-rows^T @ attn^T  (partial over this core's heads)
"""

import sys

if "/opt/trn_rl_repo" not in sys.path:
    sys.path.insert(0, "/opt/trn_rl_repo")

import numpy as np
import ml_dtypes

BF16 = ml_dtypes.bfloat16

B, S, D = 2, 2048, 1024
H, hd, C = 16, 64, 32
NCORES = 8
HPC = 4            # heads per core
SB = 512           # free-dim block (PSUM bank / max moving cols)
NSB = S // SB      # 4 seq blocks of 512
NKT = S // 128     # 16 key tiles of 128
DC = D // 128      # 8 contraction chunks for qkv
PMB = D // 128     # 8 output-row blocks for c_proj

_cache = {}


def _build():
    import os
    import concourse.bacc as bacc
    import concourse.tile as tile
    import concourse.mybir as mybir

    dt = mybir.dt
    f32, bf16 = dt.float32, dt.bfloat16
    Exp = mybir.ActivationFunctionType.Exp
    if os.environ.get("PROBE_NOEXP"):
        Exp = mybir.ActivationFunctionType.Copy
    mult = mybir.AluOpType.mult

    nc = bacc.Bacc("TRN2", target_bir_lowering=False, debug=False, num_devices=NCORES)

    hidden_t = nc.dram_tensor("hidden_t", [D, S], bf16, kind="ExternalInput")
    w_qkv = nc.dram_tensor("w_qkv", [D, 6 * 128], bf16, kind="ExternalInput")
    b_qkv = nc.dram_tensor("b_qkv", [128, 6], f32, kind="ExternalInput")
    wv = nc.dram_tensor("wv", [HPC, hd, hd], bf16, kind="ExternalInput")
    w_proj = nc.dram_tensor("w_proj", [HPC, hd, D], bf16, kind="ExternalInput")
    maskbig = nc.dram_tensor("maskbig", [128, 896], bf16, kind="ExternalInput")
    out_t = nc.dram_tensor("out_t", [D, S], bf16, kind="ExternalOutput")

    with tile.TileContext(nc) as tc:
        with (
            tc.tile_pool(name="persist", bufs=1) as pp,
            tc.tile_pool(name="work", bufs=4) as wp,
            tc.tile_pool(name="epool", bufs=36) as ep,
            tc.tile_pool(name="ostage", bufs=3) as op,
            tc.tile_pool(name="dscr", bufs=4, space="DRAM") as dr,
            tc.tile_pool(name="ps_big", bufs=4, space="PSUM") as ps_big,
            tc.tile_pool(name="ps_o", bufs=4, space="PSUM") as ps_o,
        ):
            # ---- load weights / hidden ----
            hT = []
            wq = []
            for d in range(DC):
                t = pp.tile([128, S], bf16, tag=f"hT{d}", name=f"hT{d}")
                nc.sync.dma_start(t[:], hidden_t.ap()[d * 128:(d + 1) * 128, :])
                hT.append(t)
                w = pp.tile([128, 6 * 128], bf16, tag=f"wq{d}", name=f"wq{d}")
                nc.sync.dma_start(w[:], w_qkv.ap()[d * 128:(d + 1) * 128, :])
                wq.append(w)
            bias = pp.tile([128, 6], f32, tag="bias", name="bias")
            nc.sync.dma_start(bias[:], b_qkv.ap())
            maskt = pp.tile([128, 896], bf16, tag="mask", name="maskt")
            nc.sync.dma_start(maskt[:], maskbig.ap())

            wv_t, wpj = [], []
            for h in range(HPC):
                p = (h % 2) * 64
                t = pp.tile([128, hd], bf16, tag=f"wv{h}", name=f"wv{h}")
                nc.sync.dma_start(t[p:p + 64, :], wv.ap()[h])
                wv_t.append(t)
            for p in range(2):
                t = pp.tile([128, D], bf16, tag=f"wpj{p}", name=f"wpj{p}")
                nc.sync.dma_start(t[0:hd, :], w_proj.ap()[2 * p])
                nc.sync.dma_start(t[hd:128, :], w_proj.ap()[2 * p + 1])
                wpj.append(t)

            # ---- qkv^T: 6 m-blocks (q|q, k'|k', v|v head pairs) x 4 s-blocks ----
            qq = [pp.tile([128, S], bf16, tag=f"qq{p}", name=f"qq{p}") for p in range(2)]
            kk = [pp.tile([128, S], bf16, tag=f"kk{p}", name=f"kk{p}") for p in range(2)]
            vt = [pp.tile([128, S], bf16, tag=f"vt{p}", name=f"vt{p}") for p in range(2)]
            dests = qq + kk + vt
            for sb in range(NSB):
                for mb in range(6):
                    psA = ps_big.tile([128, SB], f32, tag="psS", name="psA")
                    psB = ps_big.tile([128, SB], f32, tag="psS", name="psB")
                    for d in range(DC):
                        nc.tensor.matmul(
                            psA[:] if d % 2 == 0 else psB[:],
                            wq[d][:, mb * 128:(mb + 1) * 128],
                            hT[d][:, sb * SB:(sb + 1) * SB],
                            start=(d < 2),
                            stop=(d >= DC - 2),
                        )
                    dsl = dests[mb][:, sb * SB:(sb + 1) * SB]
                    nc.vector.tensor_scalar_add(out=dsl, in0=psA[:], scalar1=bias[:, mb:mb + 1])
                    nc.vector.tensor_tensor(dsl, dsl, psB[:], mybir.AluOpType.add)

            def rows(h):
                p = (h % 2) * 64
                return slice(p, p + 64)

            def qT(h):
                return qq[h // 2][rows(h), :]

            def kdecT(h):
                return kk[h // 2][rows(h), :]

            def vT(h):
                return vt[h // 2][rows(h), :]

            # ---- per-head v decompress (W_v folded on host) + ones column ----
            vdo = [pp.tile([128, NKT * (hd + 1)], bf16, tag=f"vdo{h}", name=f"vdo{h}") for h in range(HPC)]
            for h in range(HPC):
                nc.vector.memset(vdo[h][:], 1.0)
                for st in range(NKT):
                    ps = ps_o.tile([128, hd], f32, tag="psO", name="psC")
                    nc.tensor.matmul(
                        ps[:],
                        vT(h)[:, st * 128:(st + 1) * 128],
                        wv_t[h][rows(h), :],
                    )
                    nc.vector.tensor_copy(
                        vdo[h][:, st * (hd + 1):st * (hd + 1) + hd], ps[:]
                    )

            # ---- attention + merge ----
            # attn packed in head pairs for K=128 c_proj: tile p rows 0-63 =
            # head 2p, rows 64-127 = head 2p+1 (odd heads via DMA shift)
            attn = [pp.tile([128, S], bf16, tag=f"attn{p}", name=f"attn{p}") for p in range(2)]
            for h in range(HPC):
                for qsb in range(NSB):
                    qsl = slice(qsb * SB, (qsb + 1) * SB)
                    nkb = 4 * qsb + 4
                    psoA = ps_o.tile([hd + 1, SB], f32, tag="psO", name="psoA")
                    psoB = ps_o.tile([hd + 1, SB], f32, tag="psO", name="psoB")
                    for kb in range(nkb):
                        r = kb - 4 * qsb
                        c0 = max(r, 0) * 128  # cols < c0 are causally dead
                        psS = ps_big.tile([128, SB], f32, tag="psS", name="psS")
                        nc.tensor.matmul(
                            psS[:, c0:SB],
                            kdecT(h)[:, kb * 128:(kb + 1) * 128],
                            qT(h)[:, qsb * SB + c0:(qsb + 1) * SB],
                        )
                        e = ep.tile([128, SB], bf16, tag="E", name="e")
                        if r < 0:
                            nc.scalar.activation(e[:], psS[:], Exp)
                        else:
                            # band tile: one diagonal 128-col block, rest valid
                            c1 = c0 + 128
                            et = wp.tile([128, 128], bf16, tag="etd", name="etd")
                            nc.scalar.activation(et[:], psS[:, c0:c1], Exp)
                            nc.vector.tensor_tensor(
                                e[:, c0:c1], et[:], maskt[:, 384:512], mult
                            )
                            if c1 < SB:
                                nc.scalar.activation(e[:, c1:SB], psS[:, c1:SB], Exp)
                        pso = psoA if kb % 2 == 0 else psoB
                        av_c0 = 0 if (qsb == 0 and kb == 1) else c0
                        if av_c0 < c0:
                            nc.vector.memset(e[:, av_c0:c0], 0.0)
                        nc.tensor.matmul(
                            pso[:, av_c0:SB],
                            vdo[h][:, kb * (hd + 1):(kb + 1) * (hd + 1)],
                            e[:, av_c0:SB],
                            start=(kb < 2),
                            stop=(kb >= nkb - 2),
                        )
                    # normalize: num/den via DMA-bounced denominator broadcast
                    nsb = wp.tile([hd + 1, SB], bf16, tag="nsb", name="nsb")
                    nc.vector.tensor_copy(nsb[:], psoA[:])
                    nc.vector.tensor_tensor(nsb[:], nsb[:], psoB[:], mybir.AluOpType.add)
                    den_d = dr.tile([SB], bf16, tag="den_d", name="den_d")
                    nc.sync.dma_start(den_d[:], nsb[hd:hd + 1, :])
                    den_c = wp.tile([128, 4], bf16, tag="den_c", name="den_c")
                    nc.sync.dma_start(
                        den_c[:], den_d[:].rearrange("(p j) -> p j", p=128)
                    )
                    rec_c = wp.tile([128, 4], bf16, tag="rec_c", name="rec_c")
                    with nc.allow_low_precision(reason="softmax denom recip in bf16"):
                        nc.vector.reciprocal(rec_c[:], den_c[:])
                    rec_d = dr.tile([SB], bf16, tag="rec_d", name="rec_d")
                    nc.sync.dma_start(
                        rec_d[:].rearrange("(p j) -> p j", p=128), rec_c[:]
                    )
                    bcast = wp.tile([hd, SB], bf16, tag="bcast", name="bcast")
                    nc.sync.dma_start(
                        bcast[:], rec_d[:].unsqueeze(0).to_broadcast([hd, SB])
                    )
                    if h % 2 == 0:
                        nc.vector.tensor_tensor(
                            attn[h // 2][0:hd, qsl], nsb[0:hd, :], bcast[:], mult
                        )
                    else:
                        # odd head lands on partitions 64-127: DVE can't cross
                        # partitions, so mul into a tmp then DMA-shift
                        atmp = wp.tile([hd, SB], bf16, tag="atmp", name="atmp")
                        nc.vector.tensor_tensor(atmp[:], nsb[0:hd, :], bcast[:], mult)
                        nc.sync.dma_start(attn[h // 2][hd:128, qsl], atmp[:])
                    pace = float(os.environ.get("PACE", "0") or 0)
                    if pace > 0:
                        blk_cyc = sum(
                            (SB - max(kb - 4 * qsb, 0) * 128) * 2 + 120
                            for kb in range(nkb)
                        )
                        nc.tensor.nop(cycle_cnt=int(pace * blk_cyc))

            # ---- partial c_proj: out^T[mb*128:, sb*512:], K=128 per pair ----
            for sb in range(NSB):
                sl = slice(sb * SB, (sb + 1) * SB)
                for mb in range(PMB):
                    ps = ps_big.tile([128, SB], f32, tag="psS", name="psP")
                    for p in range(2):
                        nc.tensor.matmul(
                            ps[:],
                            wpj[p][:, mb * 128:(mb + 1) * 128],
                            attn[p][:, sl],
                            start=(p == 0),
                            stop=(p == 1),
                        )
                    stage = op.tile([128, SB], bf16, tag="stage", name="stage")
                    nc.vector.tensor_copy(stage[:], ps[:])
                    nc.sync.dma_start(out_t.ap()[mb * 128:(mb + 1) * 128, sl], stage[:])

    nc.compile()
    return nc


def _prep_inputs(hidden_states, w_attn, b_attn, wk_c, wv_c, wk_d, wv_d, w_proj):
    """Per-core input maps (host-side shard + pack + bf16 cast).

    The KV compressor is linear + low-rank, so it folds on host:
      W_k[h] = wk_c[h] @ wk_d[h] / sqrt(hd)  -> folded into w_attn k-columns
      W_v[h] = wv_c[h] @ wv_d[h]             -> single on-device matmul
    """
    hidden_T = [np.ascontiguousarray(hidden_states[b].T).astype(BF16) for b in range(B)]
    Wk = np.einsum("hdc,hce->hde", wk_c.astype(np.float64),
                   wk_d.astype(np.float64)) * (1.0 / np.sqrt(hd))  # [H,hd,hd]
    Wv = np.einsum("hdc,hce->hde", wv_c.astype(np.float64),
                   wv_d.astype(np.float64))                        # [H,hd,hd]
    wq_h = lambda h: w_attn[:, h * hd:(h + 1) * hd]
    wk_h = lambda h: (w_attn[:, D + h * hd:D + (h + 1) * hd].astype(np.float64)
                      @ Wk[h]).astype(np.float32)
    wv_h = lambda h: w_attn[:, 2 * D + h * hd:2 * D + (h + 1) * hd]
    bq_h = lambda h: b_attn[h * hd:(h + 1) * hd]
    bk_h = lambda h: (b_attn[D + h * hd:D + (h + 1) * hd].astype(np.float64)
                      @ Wk[h]).astype(np.float32)
    bv_h = lambda h: b_attn[2 * D + h * hd:2 * D + (h + 1) * hd]
    in_maps = []
    for c in range(NCORES):
        b = c // 4
        hs = list(range((c % 4) * HPC, (c % 4) * HPC + HPC))
        # m-blocks: [q0|q1], [q2|q3], [k'0|k'1], [k'2|k'3], [v0|v1], [v2|v3]
        cols, bcols = [], []
        for fn, bfn in ((wq_h, bq_h), (wk_h, bk_h), (wv_h, bv_h)):
            for h in hs:
                cols.append(fn(h))
                bcols.append(bfn(h))
        w_qkv_l = np.concatenate(cols, axis=1).astype(BF16)        # [1024, 768]
        b_qkv_l = (
            np.concatenate(bcols).astype(np.float32).reshape(6, 128).T.copy()
        )                                                          # [128, 6]
        k = np.arange(128).reshape(128, 1)
        cgrid = np.arange(896).reshape(1, 896)
        mask = (k <= cgrid - 384).astype(BF16)
        in_maps.append(
            {
                "hidden_t": hidden_T[b],
                "w_qkv": w_qkv_l,
                "b_qkv": b_qkv_l,
                "wv": Wv[hs].astype(BF16),
                "w_proj": np.stack(
                    [w_proj[h * hd:(h + 1) * hd, :] for h in hs]
                ).astype(BF16),
                "maskbig": np.ascontiguousarray(mask),
            }
        )
    return in_maps


def kernel(
    hidden_states,
    w_attn,
    b_attn,
    w_proj,
    b_proj,
    wk_c,
    wv_c,
    wk_d,
    wv_d,
    _trace=False,
):
    from concourse.bass_utils import run_bass_kernel_spmd

    if "nc" not in _cache:
        _cache["nc"] = _build()
    nc = _cache["nc"]

    in_maps = _prep_inputs(
        np.asarray(hidden_states),
        np.asarray(w_attn),
        np.asarray(b_attn),
        np.asarray(wk_c),
        np.asarray(wv_c),
        np.asarray(wk_d),
        np.asarray(wv_d),
        np.asarray(w_proj),
    )
    res = run_bass_kernel_spmd(
        nc, in_maps, core_ids=list(range(NCORES)), trace=_trace
    )
    out = np.empty((B, S, D), np.float32)
    for b in range(B):
        acc = np.zeros((D, S), np.float32)
        for c in range(4 * b, 4 * b + 4):
            acc += res.results[c]["out_t"].astype(np.float32)
        out[b] = acc.T + np.asarray(b_proj, np.float32)
    if _trace:
        _cache["last_exec_time_ns"] = res.exec_time_ns
        _cache["last_results"] = res
    return out
